# revision 6
# baseline (speedup 1.0000x reference)
"""Trainium2 Bass kernel for a ChannelAttention module.

Reference computation (per row b of B = 2048 rows, each row is (n=64, c=512)):
    y  = mean_c x                      # (B, 64)
    lr = y @ w1.T + b1                 # (B, 32)
    f1 = lr @ mb                       # (B, 128)
    at = softmax(f1 / sqrt(32))        # (B, 128)
    y1 = at @ mb.T                     # (B, 32)
    y2 = sigmoid(y1 @ w2.T + b2)       # (B, 64)
    out = x * y2[..., None]

Memory-bound: 256 MiB in + 256 MiB out. Strategy: data-parallel over 8 cores
(256 rows each), single streaming pass per core. The two inner linears fold
host-side into two small fused matrices so the on-chip MLP is:
    f1_raw = y_sum @ A          A = (w1.T @ mb) / 512          [64, 128]
    e      = exp(f1_raw*s + be) be = (b1 @ mb) * s, s=32^-0.5  [128, 1]
    [z|S]  = Daug.T @ e         Daug = [(w2 @ mb).T | ones]    [128, 65]
    y2     = sigmoid(z / S + b2)
(softmax max-subtraction is skipped: |f1*s| < ~3 for these magnitudes, and the
result is mathematically identical.)

SBUF layout: x streamed as [128, 512] tiles = 2 rows per tile, partition
p = r*64 + j (r = row parity, j = channel). The c-reduction lands in
y_coll[128, G]; its partition halves ARE the transposed-MLP operand
yT [j, col] for even/odd rows, so no on-chip transpose is ever needed.
"""

import os
import sys

import numpy as np

for _p in ("/opt/trn_rl_repo",):
    if _p not in sys.path:
        sys.path.insert(0, _p)

from contextlib import ExitStack

from concourse import bacc, mybir, tile
from concourse.bass_utils import run_bass_kernel_spmd

N_CORES = 8
ROWS = 2048              # total B rows
C = 512
N = 64
P = 128
TILES = (ROWS // N_CORES) // 2   # 128 [128, 512] tiles per core, 2 rows each
G = 16                           # tiles per MLP chunk
FP = mybir.dt.float32
SCALE = float(32 ** -0.5)

_CACHED = None
LAST_RESULTS = None  # BassKernelResults of the most recent kernel() call


def _build_module(tiles=TILES, g=G, repeat=1):
    """repeat>1 wraps the streaming pass in an on-device For_i loop —
    used only for differential exec-time measurement (dispatch overhead
    cancels between two repeat counts)."""
    nchunk = tiles // g
    nc = bacc.Bacc("TRN2", target_bir_lowering=False, debug=False)

    x_d = nc.dram_tensor("x", [tiles, P, C], FP, kind="ExternalInput")
    a_d = nc.dram_tensor("amat", [N, P], FP, kind="ExternalInput")
    be_d = nc.dram_tensor("bexp", [P, 1], FP, kind="ExternalInput")
    dg_d = nc.dram_tensor("daug", [P, N + 1], FP, kind="ExternalInput")
    b2_d = nc.dram_tensor("b2", [N, 1], FP, kind="ExternalInput")
    o_d = nc.dram_tensor("out", [tiles, P, C], FP, kind="ExternalOutput")

    with tile.TileContext(nc) as tc, ExitStack() as ctx:
        const = ctx.enter_context(tc.tile_pool(name="const", bufs=1))
        xp = ctx.enter_context(tc.tile_pool(name="xp", bufs=2 * g))
        yp = ctx.enter_context(tc.tile_pool(name="yp", bufs=2))
        sp = ctx.enter_context(tc.tile_pool(name="sp", bufs=2))
        svp = ctx.enter_context(tc.tile_pool(name="svp", bufs=2 * g))
        pp = ctx.enter_context(tc.tile_pool(name="pp", bufs=2, space="PSUM"))

        a_sb = const.tile([N, P], FP)
        nc.sync.dma_start(a_sb[:], a_d[:])
        be_sb = const.tile([P, 1], FP)
        nc.sync.dma_start(be_sb[:], be_d[:])
        dg_sb = const.tile([P, N + 1], FP)
        nc.sync.dma_start(dg_sb[:], dg_d[:])
        b2_sb = const.tile([N, 1], FP)
        nc.sync.dma_start(b2_sb[:], b2_d[:])
        ones_sb = const.tile([1, N], FP)
        nc.vector.memset(ones_sb[:], 1.0)

        loop_cm = tc.For_i(0, repeat, 1) if repeat > 1 else None
        if loop_cm is not None:
            loop_cm.__enter__()

        for ch in range(nchunk):
            y_coll = yp.tile([P, g], FP)
            xts = []
            for i in range(g):
                t = ch * g + i
                xt = xp.tile([P, C], FP)
                nc.sync.dma_start(xt[:], x_d[t])
                nc.vector.reduce_sum(
                    y_coll[:, i : i + 1], xt[:], axis=mybir.AxisListType.X
                )
                xts.append(xt)

            # y_coll halves are yT for even/odd rows: pack to [64, 2g]
            y_all = sp.tile([N, 2 * g], FP)
            nc.vector.tensor_copy(y_all[:, 0:g], y_coll[0:N, :])
            nc.vector.tensor_copy(y_all[:, g : 2 * g], y_coll[N:P, :])

            f1 = pp.tile([P, 2 * g], FP)
            nc.tensor.matmul(f1[:], a_sb[:], y_all[:])
            e_sb = sp.tile([P, 2 * g], FP)
            nc.scalar.activation(
                e_sb[:], f1[:], mybir.ActivationFunctionType.Exp,
                bias=be_sb[:], scale=SCALE,
            )
            zs = pp.tile([N + 1, 2 * g], FP)
            nc.tensor.matmul(zs[:], dg_sb[:], e_sb[:])
            rs = sp.tile([1, 2 * g], FP)
            nc.vector.reciprocal(rs[:], zs[N : N + 1, :])
            rb = pp.tile([N, 2 * g], FP)
            nc.tensor.matmul(rb[:], ones_sb[:], rs[:])
            rb_sb = sp.tile([N, 2 * g], FP)
            nc.scalar.copy(rb_sb[:], rb[:])
            zn = sp.tile([N, 2 * g], FP)
            nc.vector.tensor_mul(zn[:], zs[0:N, :], rb_sb[:])
            y2 = sp.tile([N, 2 * g], FP)
            nc.scalar.activation(
                y2[:], zn[:], mybir.ActivationFunctionType.Sigmoid, bias=b2_sb[:]
            )

            for i in range(g):
                t = ch * g + i
                sv = svp.tile([P, 1], FP)
                nc.vector.tensor_copy(sv[0:N, :], y2[:, i : i + 1])
                nc.vector.tensor_copy(sv[N:P, :], y2[:, g + i : g + i + 1])
                xt = xts[i]
                nc.scalar.activation(
                    xt[:], xt[:], mybir.ActivationFunctionType.Copy, scale=sv[:]
                )
                nc.sync.dma_start(o_d[t], xt[:])

        if loop_cm is not None:
            loop_cm.__exit__(None, None, None)

    nc.compile()
    return nc


def _prep_weights(w1, b1, w2, b2, mb):
    w1 = np.asarray(w1, np.float64)
    b1 = np.asarray(b1, np.float64)
    w2 = np.asarray(w2, np.float64)
    b2 = np.asarray(b2, np.float64)
    mb = np.asarray(mb, np.float64)
    a = np.ascontiguousarray(((w1.T @ mb) / C).astype(np.float32))
    be = np.ascontiguousarray(((b1 @ mb) * SCALE).astype(np.float32).reshape(P, 1))
    dg = np.concatenate([(w2 @ mb).T, np.ones((P, 1))], axis=1)
    dg = np.ascontiguousarray(dg.astype(np.float32))
    b2c = np.ascontiguousarray(b2.astype(np.float32).reshape(N, 1))
    return a, be, dg, b2c


def kernel(x, w1, b1, w2, b2, mb):
    global _CACHED
    x = np.ascontiguousarray(np.asarray(x, np.float32))
    b, Nn, Nwin, p, n, c = x.shape
    a, be, dg, b2c = _prep_weights(w1, b1, w2, b2, mb)

    if _CACHED is None:
        _CACHED = _build_module()
    nc = _CACHED

    xs = x.reshape(N_CORES, TILES, P, C)
    in_maps = [
        {"x": xs[i], "amat": a, "bexp": be, "daug": dg, "b2": b2c}
        for i in range(N_CORES)
    ]
    global LAST_RESULTS
    LAST_RESULTS = run_bass_kernel_spmd(
        nc, in_maps, core_ids=list(range(N_CORES)),
        trace=bool(os.environ.get("KERNEL_TRACE")),
    )
    res = LAST_RESULTS.results
    out = np.stack([r["out"] for r in res], axis=0)
    return out.reshape(b, Nn, Nwin, p, n, c)


if __name__ == "__main__":
    xt = np.random.randn(2, 16, 16, 4, 64, 512).astype(np.float32)
    w1t = (np.random.randn(32, 64) * 0.1).astype(np.float32)
    b1t = (np.random.randn(32) * 0.1).astype(np.float32)
    w2t = (np.random.randn(64, 32) * 0.1).astype(np.float32)
    b2t = (np.random.randn(64) * 0.1).astype(np.float32)
    mbt = np.random.randn(32, 128).astype(np.float32)
    o = kernel(xt, w1t, b1t, w2t, b2t, mbt)
    print(o.shape, o.dtype)
